# revision 24
# baseline (speedup 1.0000x reference)
"""RGCN (basis-decomposition, one-hot features) message passing on 8 trn2 NeuronCores.

Math (per reference):
    weight[r] = sum_b comp[r,b] * basis[b]          # [R, N, OUT]
    msg_e     = weight[edge_type_e, src_e]          # [E, OUT]
    agg       = segment_sum(msg, dst) / max(cnt, 1) # mean aggregation
    out       = log_softmax(relu(agg + root + bias), axis=1)

Distribution strategy: shard edges by DESTINATION node range across the 8
cores (each core owns N/8 output rows), so no collectives are needed.  Each
core gathers, per edge, the 5 basis rows of its src node (host-side layout
`basisR[n] = basis[:, n, :]` flattened, 6000B contiguous per node) via
indirect DMA, then scatter-accumulates messages into a per-128-node-tile
PSUM accumulator using TensorE matmuls with on-device-built one-hot
indicator matrices scaled by comp[edge_type] coefficients:

    psum[tile] += (onehot(dst_local) * comp[type, b]).T @ basisR_rows[:, b, :]

The comp coefficients are fetched per-edge from a tiny DRAM table with a
second indirect DMA.  The epilogue (mean, +root, +bias, relu, log_softmax)
runs fused per tile on DVE/ACT and streams the final rows out.

Host-side work is limited to index metadata (sharding, grouping edges by
destination tile, padding, bincount) and data layout (transpose/pad/
replicate of input tensors).  All floating-point math runs on device.
"""

import math
from contextlib import ExitStack
from dataclasses import dataclass, field

import numpy as np

import ml_dtypes

import concourse.bacc as bacc
import concourse.bass as bass
import concourse.mybir as mybir
import concourse.tile as tile
from concourse.bass_utils import run_bass_kernel_spmd

F32 = mybir.dt.float32
BF16 = mybir.dt.bfloat16
I32 = mybir.dt.int32
P = 128  # SBUF partitions


@dataclass
class Cfg:
    n_nodes: int = 50000
    n_rel: int = 5
    n_bases: int = 5
    out_dim: int = 300
    n_cores: int = 8
    g_blk: int = 8         # gather-group size in 128-edge blocks
    tdt: str = "bf16"      # gather-table dtype: "bf16" or "f32"
    enable_asserts: bool = False

    @property
    def row(self) -> int:  # elements per basisR row
        return self.n_bases * self.out_dim

    @property
    def npc(self) -> int:  # nodes per core, multiple of 128
        return ((self.n_nodes + self.n_cores - 1) // self.n_cores + P - 1) // P * P

    @property
    def n_tiles(self) -> int:
        return self.npc // P


# ----------------------------------------------------------------------------
# Host-side prep: index metadata + layout only (no float math on tensor data)
# ----------------------------------------------------------------------------

def host_prep(cfg: Cfg, edge_index, edge_type, basis, root, bias, comp):
    src = np.asarray(edge_index[0]).astype(np.int64)
    dst = np.asarray(edge_index[1]).astype(np.int64)
    etype = np.asarray(edge_type).astype(np.int64)
    basis = np.asarray(basis, dtype=np.float32)
    root = np.asarray(root, dtype=np.float32)
    bias = np.asarray(bias, dtype=np.float32)
    comp = np.asarray(comp, dtype=np.float32)

    N, R, B, OUT = cfg.n_nodes, cfg.n_rel, cfg.n_bases, cfg.out_dim
    NPC, T = cfg.npc, cfg.n_tiles

    # layout: basisR[n] = basis[:, n, :] flattened -> [N, B*OUT], contiguous row
    basisR = np.ascontiguousarray(basis.transpose(1, 0, 2).reshape(N, B * OUT))
    if cfg.tdt == "bf16":
        basisR = basisR.astype(ml_dtypes.bfloat16)
    biast = np.ascontiguousarray(np.broadcast_to(bias[None, :], (P, OUT)))

    core_of = dst // NPC
    per_core = []
    tile_counts = np.zeros((cfg.n_cores, T), dtype=np.int64)
    for c in range(cfg.n_cores):
        m = core_of == c
        s_c, t_c, dl_c = src[m], etype[m], dst[m] - c * NPC
        order = np.argsort(dl_c, kind="stable")
        s_c, t_c, dl_c = s_c[order], t_c[order], dl_c[order]
        tid = dl_c // P
        tile_counts[c] = np.bincount(tid, minlength=T)
        per_core.append((s_c, t_c, dl_c, tid))

    # blocks per tile: shared static structure across all cores
    B_t = [max(1, int(math.ceil(tile_counts[:, t].max() / P))) for t in range(T)]
    NB = sum(B_t)

    in_maps = []
    for c in range(cfg.n_cores):
        s_c, t_c, dl_c, tid = per_core[c]
        srcs = np.zeros((NB * P,), dtype=np.int32)
        typs = np.zeros((NB * P,), dtype=np.int32)
        dstf = np.full((NB * P,), -1.0, dtype=np.float32)
        off = 0
        pos = np.concatenate([[0], np.cumsum(np.bincount(tid, minlength=T))])
        for t in range(T):
            a, b = pos[t], pos[t + 1]
            n = b - a
            srcs[off : off + n] = s_c[a:b]
            typs[off : off + n] = t_c[a:b]
            dstf[off : off + n] = (dl_c[a:b] - t * P).astype(np.float32)
            off += B_t[t] * P
        # edge slot e within its tile run -> (block j=e//P, lane p=e%P);
        # device arrays are [P, NB] with [p, j] = slot j*P+p
        srcs2 = srcs.reshape(NB, P).T.copy()
        typs2 = typs.reshape(NB, P).T.copy()
        dstf2 = dstf.reshape(NB, P).T.copy()

        cnt = np.zeros((NPC,), dtype=np.float32)
        real_dst = dl_c  # all real edges for this core
        np.add.at(cnt, real_dst, 1.0)
        cnt2 = cnt.reshape(T, P).T.copy()

        rootp = np.zeros((NPC, OUT), dtype=np.float32)
        lo, hi = c * NPC, min((c + 1) * NPC, N)
        if hi > lo:
            rootp[: hi - lo] = root[lo:hi]

        # pack every per-core constant into ONE f32 buffer (ints bitcast) so
        # the device needs a single const DMA -> a single producer semaphore.
        # iota is stored in the gather-table dtype (two bf16 per f32 slot).
        if cfg.tdt == "bf16":
            iota = np.ascontiguousarray(np.broadcast_to(
                np.arange(P, dtype=ml_dtypes.bfloat16)[None, :], (P, P))
            ).view(np.float32)
        else:
            iota = np.broadcast_to(
                np.arange(P, dtype=np.float32)[None, :], (P, P))
        consts = np.concatenate(
            [
                srcs2.view(np.float32),
                typs2.astype(np.float32),  # float copy for is_equal matching
                dstf2,
                cnt2,
                iota,
                biast,
            ],
            axis=1,
        ).copy()

        in_maps.append(dict(basisR=basisR, consts=consts, rootp=rootp))
    return B_t, in_maps


# ----------------------------------------------------------------------------
# Device program
# ----------------------------------------------------------------------------

def build_program(cfg: Cfg, B_t, comp):
    N, R, OUT, ROW = cfg.n_nodes, cfg.n_rel, cfg.out_dim, cfg.row
    NPC, T = cfg.npc, cfg.n_tiles
    NB = sum(B_t)
    comp = np.asarray(comp, dtype=np.float32)

    nc = bacc.Bacc(
        "TRN2",
        target_bir_lowering=False,
        debug=False,
        enable_asserts=cfg.enable_asserts,
        num_devices=cfg.n_cores,
    )
    TDT = BF16 if cfg.tdt == "bf16" else F32
    IOTA_W = P // 2 if cfg.tdt == "bf16" else P  # f32 slots holding iota
    W = 3 * NB + T + IOTA_W + OUT  # packed const width
    basisR = nc.dram_tensor("basisR", [N, ROW], TDT, kind="ExternalInput").ap()
    consts = nc.dram_tensor("consts", [P, W], F32, kind="ExternalInput").ap()
    rootp = nc.dram_tensor("rootp", [NPC, OUT], F32, kind="ExternalInput").ap()
    out = nc.dram_tensor("out", [NPC, OUT], F32, kind="ExternalOutput").ap()

    eq, mul, add = mybir.AluOpType.is_equal, mybir.AluOpType.mult, mybir.AluOpType.add

    with tile.TileContext(nc) as tc, ExitStack() as ctx:
        cpool = ctx.enter_context(tc.tile_pool(name="const", bufs=1))
        gpool = ctx.enter_context(tc.tile_pool(name="g", bufs=6))
        lpool = ctx.enter_context(tc.tile_pool(name="lhs", bufs=3))
        ppool = ctx.enter_context(tc.tile_pool(name="ps", bufs=2, space="PSUM"))
        rpool = ctx.enter_context(tc.tile_pool(name="root", bufs=3))
        epool = ctx.enter_context(tc.tile_pool(name="epi", bufs=2))

        consts_sb = cpool.tile([P, W], F32)
        nc.sync.dma_start(consts_sb[:], consts[:, :])
        srcs_sb = consts_sb[:, 0:NB].bitcast(I32)
        typesf_sb = consts_sb[:, NB : 2 * NB]
        dstf_sb = consts_sb[:, 2 * NB : 3 * NB]
        cnt_sb = consts_sb[:, 3 * NB : 3 * NB + T]
        iota_t = consts_sb[:, 3 * NB + T : 3 * NB + T + IOTA_W]
        if cfg.tdt == "bf16":
            iota_t = iota_t.bitcast(BF16)
        biast_sb = consts_sb[:, 3 * NB + T + IOTA_W : 3 * NB + T + IOTA_W + OUT]
        rcnt_sb = cpool.tile([P, T], F32)
        nc.vector.tensor_scalar_max(rcnt_sb[:], cnt_sb, 1.0)
        nc.vector.reciprocal(rcnt_sb[:], rcnt_sb[:])

        # Per-edge basis coefficients computed once on-device from edge types:
        # c_all[e, b] = comp[type_e, b] = sum_r (type_e == r) * comp[r, b];
        # comp values are baked in as immediates (known at trace time).
        c_all = cpool.tile([P, R, NB], F32)
        ctmp = cpool.tile([P, NB], F32)
        for b in range(R):
            for r in range(R):
                dst_ap = c_all[:, b, :] if r == 0 else ctmp[:]
                nc.vector.tensor_scalar(
                    out=dst_ap, in0=typesf_sb, scalar1=float(r),
                    scalar2=float(comp[r, b]), op0=eq, op1=mul,
                )
                if r > 0:
                    nc.vector.tensor_tensor(
                        out=c_all[:, b, :], in0=c_all[:, b, :], in1=ctmp[:],
                        op=add,
                    )

        jglob = 0
        for t in range(T):
            Bt = B_t[t]
            psum_t = ppool.tile([P, OUT], F32)
            root_t = rpool.tile([P, OUT], F32)
            nc.sync.dma_start(root_t[:], rootp[t * P : (t + 1) * P, :])
            first = True
            for j in range(Bt):
                col = jglob + j
                gt = gpool.tile([P, ROW], TDT, tag="g")
                nc.gpsimd.indirect_dma_start(
                    out=gt[:, :], out_offset=None,
                    in_=basisR[:, :],
                    in_offset=bass.IndirectOffsetOnAxis(
                        ap=srcs_sb[:, col : col + 1], axis=0),
                )
                lt = lpool.tile([P, R, P], TDT, tag="l")
                for b in range(R):
                    # lhsT_b[e, nd] = (iota[nd] == dst_e) * comp[type_e, b]
                    nc.vector.tensor_scalar(
                        out=lt[:, b, :], in0=iota_t[:],
                        scalar1=dstf_sb[:, col : col + 1],
                        scalar2=c_all[:, b, col : col + 1],
                        op0=eq, op1=mul,
                    )
                    nc.tensor.matmul(
                        psum_t[:],
                        lhsT=lt[:, b, :],
                        rhs=gt[:, b * OUT : (b + 1) * OUT],
                        start=first,
                        stop=(j == Bt - 1 and b == R - 1),
                    )
                    first = False
            jglob += Bt

            # epilogue: mean, +root, +bias, relu, log_softmax, store
            h = epool.tile([P, OUT], F32, tag="h")
            nc.vector.tensor_scalar(
                out=h[:], in0=psum_t[:], scalar1=rcnt_sb[:, t : t + 1],
                scalar2=None, op0=mul,
            )
            nc.vector.tensor_tensor(out=h[:], in0=h[:], in1=root_t[:], op=add)
            nc.vector.tensor_tensor(out=h[:], in0=h[:], in1=biast_sb[:], op=add)
            nc.vector.tensor_scalar_max(h[:], h[:], 0.0)
            mx = epool.tile([P, 2], F32, tag="mx")
            nc.vector.tensor_reduce(
                out=mx[:, 0:1], in_=h[:], axis=mybir.AxisListType.X,
                op=mybir.AluOpType.max, negate=True,
            )
            ex = epool.tile([P, OUT], F32, tag="ex")
            nc.scalar.activation(
                out=ex[:], in_=h[:], func=mybir.ActivationFunctionType.Exp,
                bias=mx[:, 0:1], scale=1.0, accum_out=mx[:, 1:2],
            )
            ln = epool.tile([P, 2], F32, tag="ln")
            nc.scalar.activation(
                out=ln[:, 0:1], in_=mx[:, 1:2], func=mybir.ActivationFunctionType.Ln,
            )
            tot = epool.tile([P, 2], F32, tag="tot")
            nc.vector.tensor_tensor(
                out=tot[:, 0:1], in0=mx[:, 0:1], in1=ln[:, 0:1],
                op=mybir.AluOpType.subtract,
            )
            o = epool.tile([P, OUT], F32, tag="o")
            nc.vector.tensor_scalar(
                out=o[:], in0=h[:], scalar1=tot[:, 0:1], scalar2=None,
                op0=add,
            )
            nc.sync.dma_start(out[t * P : (t + 1) * P, :], o[:])
    nc.compile()
    return nc


# ----------------------------------------------------------------------------
# Entry point
# ----------------------------------------------------------------------------

def _run(cfg: Cfg, inputs: dict, trace: bool = False):
    B_t, in_maps = host_prep(
        cfg,
        inputs["edge_index"], inputs["edge_type"], inputs["basis"],
        inputs["root"], inputs["bias"], inputs["comp"],
    )
    nc = build_program(cfg, B_t, inputs["comp"])
    res = run_bass_kernel_spmd(
        nc, in_maps, core_ids=list(range(cfg.n_cores)), trace=trace,
    )
    parts = [res.results[c]["out"] for c in range(cfg.n_cores)]
    full = np.concatenate(parts, axis=0)[: cfg.n_nodes]
    return np.ascontiguousarray(full.astype(np.float32)), res


def kernel(**inputs) -> np.ndarray:
    cfg = Cfg()
    out, _ = _run(cfg, inputs)
    return out


# revision 28
# speedup vs baseline: 7.9368x; 7.9368x over previous
"""RGCN (basis-decomposition, one-hot features) message passing on 8 trn2 NeuronCores.

Math (per reference):
    weight[r] = sum_b comp[r,b] * basis[b]          # [R, N, OUT]
    msg_e     = weight[edge_type_e, src_e]          # [E, OUT]
    agg       = segment_sum(msg, dst) / max(cnt, 1) # mean aggregation
    out       = log_softmax(relu(agg + root + bias), axis=1)

Distribution strategy: shard edges by DESTINATION node range across the 8
cores (each core owns N/8 output rows), so no collectives are needed.  Each
core gathers, per edge, the 5 basis rows of its src node (host-side layout
`basisR[n] = basis[:, n, :]` flattened, 6000B contiguous per node) via
indirect DMA, then scatter-accumulates messages into a per-128-node-tile
PSUM accumulator using TensorE matmuls with on-device-built one-hot
indicator matrices scaled by comp[edge_type] coefficients:

    psum[tile] += (onehot(dst_local) * comp[type, b]).T @ basisR_rows[:, b, :]

The comp coefficients are fetched per-edge from a tiny DRAM table with a
second indirect DMA.  The epilogue (mean, +root, +bias, relu, log_softmax)
runs fused per tile on DVE/ACT and streams the final rows out.

Host-side work is limited to index metadata (sharding, grouping edges by
destination tile, padding, bincount) and data layout (transpose/pad/
replicate of input tensors).  All floating-point math runs on device.
"""

import math
from contextlib import ExitStack
from dataclasses import dataclass, field

import numpy as np

import ml_dtypes

import concourse.bacc as bacc
import concourse.bass as bass
import concourse.mybir as mybir
import concourse.tile as tile
from concourse.bass_utils import run_bass_kernel_spmd

F32 = mybir.dt.float32
BF16 = mybir.dt.bfloat16
I32 = mybir.dt.int32
P = 128  # SBUF partitions


@dataclass
class Cfg:
    n_nodes: int = 50000
    n_rel: int = 5
    n_bases: int = 5
    out_dim: int = 300
    n_cores: int = 8
    g_blk: int = 8         # gather-group size in 128-edge blocks
    tdt: str = "bf16"      # gather-table dtype: "bf16" or "f32"
    gather: str = "ant"    # "ant" (dma_gather, int16 idx) or "indirect"
    enable_asserts: bool = False

    @property
    def rowp(self) -> int:  # padded basisR row (ant path needs 256B multiple)
        if self.gather == "ant":
            esz = 2 if self.tdt == "bf16" else 4
            per = 256 // math.gcd(256, esz)
            return ((self.row + per - 1) // per) * per
        return self.row

    @property
    def half(self) -> int:  # src-range half size for int16 gather indices
        return (self.n_nodes + 1) // 2

    @property
    def row(self) -> int:  # elements per basisR row
        return self.n_bases * self.out_dim

    @property
    def npc(self) -> int:  # nodes per core, multiple of 128
        return ((self.n_nodes + self.n_cores - 1) // self.n_cores + P - 1) // P * P

    @property
    def n_tiles(self) -> int:
        return self.npc // P


# ----------------------------------------------------------------------------
# Host-side prep: index metadata + layout only (no float math on tensor data)
# ----------------------------------------------------------------------------

def host_prep(cfg: Cfg, edge_index, edge_type, basis, root, bias, comp):
    src = np.asarray(edge_index[0]).astype(np.int64)
    dst = np.asarray(edge_index[1]).astype(np.int64)
    etype = np.asarray(edge_type).astype(np.int64)
    basis = np.asarray(basis, dtype=np.float32)
    root = np.asarray(root, dtype=np.float32)
    bias = np.asarray(bias, dtype=np.float32)
    comp = np.asarray(comp, dtype=np.float32)

    N, R, B, OUT = cfg.n_nodes, cfg.n_rel, cfg.n_bases, cfg.out_dim
    NPC, T = cfg.npc, cfg.n_tiles

    # layout: basisR[n] = basis[:, n, :] flattened -> [N, rowp], contiguous row
    basisR = np.zeros((N, cfg.rowp), dtype=np.float32)
    basisR[:, : B * OUT] = basis.transpose(1, 0, 2).reshape(N, B * OUT)
    if cfg.tdt == "bf16":
        basisR = basisR.astype(ml_dtypes.bfloat16)
    biast = np.ascontiguousarray(np.broadcast_to(bias[None, :], (P, OUT)))

    ant = cfg.gather == "ant"
    HALF = cfg.half
    n_h = 2 if ant else 1

    core_of = dst // NPC
    per_core = []
    counts = np.zeros((cfg.n_cores, T, n_h), dtype=np.int64)
    for c in range(cfg.n_cores):
        m = core_of == c
        s_c, t_c, dl_c = src[m], etype[m], dst[m] - c * NPC
        tid = dl_c // P
        hid = (s_c // HALF) if ant else np.zeros_like(s_c)
        # group edges by (tile, half)
        order = np.argsort(tid * n_h + hid, kind="stable")
        s_c, t_c, dl_c = s_c[order], t_c[order], dl_c[order]
        tid, hid = tid[order], hid[order]
        for t in range(T):
            for h in range(n_h):
                counts[c, t, h] = np.count_nonzero((tid == t) & (hid == h))
        per_core.append((s_c, t_c, dl_c))

    # cells: (tile, half, n_blocks) — shared static structure across cores
    cells = []
    for t in range(T):
        tile_cells = []
        for h in range(n_h):
            Bc = int(math.ceil(counts[:, t, h].max() / P))
            if Bc > 0:
                tile_cells.append((t, h, Bc))
        if not tile_cells:
            tile_cells.append((t, 0, 1))  # empty tile still needs psum zeroing
        cells.extend(tile_cells)
    NB = sum(Bc for (_, _, Bc) in cells)

    in_maps = []
    for c in range(cfg.n_cores):
        s_c, t_c, dl_c = per_core[c]
        srcs = np.zeros((NB * P,), dtype=np.int64)
        typs = np.zeros((NB * P,), dtype=np.int64)
        dstf = np.full((NB * P,), -1.0, dtype=np.float32)
        pos = np.concatenate(
            [[0], np.cumsum(counts[c].reshape(-1))]
        )  # prefix over (t, h) grid
        off = 0
        for (t, h, Bc) in cells:
            gi = t * n_h + h
            a, b = pos[gi], pos[gi + 1]
            n = b - a
            srcs[off : off + n] = s_c[a:b] - (h * HALF if ant else 0)
            typs[off : off + n] = t_c[a:b]
            dstf[off : off + n] = (dl_c[a:b] - t * P).astype(np.float32)
            off += Bc * P
        # edge slot e within its cell run -> (block j=e//P, lane p=e%P);
        # device arrays are [P, NB] with [p, j] = slot j*P+p
        typs2 = typs.reshape(NB, P).T.astype(np.float32)
        dstf2 = dstf.reshape(NB, P).T.copy()

        if ant:
            # int16 indices wrapped in 16 partitions, replicated to 128
            idx16 = np.ascontiguousarray(
                np.tile(srcs.astype(np.int16).reshape(NB * 8, 16).T, (8, 1))
            )  # [128, NB*8] int16
            idx_cols = idx16.view(np.float32)  # [128, NB*4]
        else:
            idx_cols = srcs.astype(np.int32).reshape(NB, P).T.view(np.float32)

        cnt = np.zeros((NPC,), dtype=np.float32)
        np.add.at(cnt, dl_c, 1.0)
        cnt2 = cnt.reshape(T, P).T.copy()

        rootp = np.zeros((NPC, OUT), dtype=np.float32)
        lo, hi = c * NPC, min((c + 1) * NPC, N)
        if hi > lo:
            rootp[: hi - lo] = root[lo:hi]

        # pack every per-core constant into ONE f32 buffer (ints bitcast) so
        # the device needs a single const DMA -> a single producer semaphore.
        # iota is stored in the gather-table dtype (two bf16 per f32 slot).
        if cfg.tdt == "bf16":
            iota = np.ascontiguousarray(np.broadcast_to(
                np.arange(P, dtype=ml_dtypes.bfloat16)[None, :], (P, P))
            ).view(np.float32)
        else:
            iota = np.broadcast_to(
                np.arange(P, dtype=np.float32)[None, :], (P, P))
        consts = np.concatenate(
            [
                idx_cols,
                typs2,  # float copy for is_equal matching
                dstf2,
                cnt2,
                iota,
                biast,
            ],
            axis=1,
        ).copy()

        in_maps.append(dict(basisR=basisR, consts=consts, rootp=rootp))
    return cells, in_maps


# ----------------------------------------------------------------------------
# Device program
# ----------------------------------------------------------------------------

def build_program(cfg: Cfg, cells, comp):
    N, R, OUT, ROW = cfg.n_nodes, cfg.n_rel, cfg.out_dim, cfg.rowp
    NPC, T = cfg.npc, cfg.n_tiles
    ant = cfg.gather == "ant"
    HALF = cfg.half
    NB = sum(Bc for (_, _, Bc) in cells)
    IDXW = NB * 4 if ant else NB  # f32 cols holding gather indices
    comp = np.asarray(comp, dtype=np.float32)

    nc = bacc.Bacc(
        "TRN2",
        target_bir_lowering=False,
        debug=False,
        enable_asserts=cfg.enable_asserts,
        num_devices=cfg.n_cores,
    )
    TDT = BF16 if cfg.tdt == "bf16" else F32
    I16 = mybir.dt.int16
    IOTA_W = P // 2 if cfg.tdt == "bf16" else P  # f32 slots holding iota
    W = IDXW + 2 * NB + T + IOTA_W + OUT  # packed const width
    basisR = nc.dram_tensor("basisR", [N, ROW], TDT, kind="ExternalInput").ap()
    consts = nc.dram_tensor("consts", [P, W], F32, kind="ExternalInput").ap()
    rootp = nc.dram_tensor("rootp", [NPC, OUT], F32, kind="ExternalInput").ap()
    out = nc.dram_tensor("out", [NPC, OUT], F32, kind="ExternalOutput").ap()

    eq, mul, add = mybir.AluOpType.is_equal, mybir.AluOpType.mult, mybir.AluOpType.add

    with tile.TileContext(nc) as tc, ExitStack() as ctx:
        cpool = ctx.enter_context(tc.tile_pool(name="const", bufs=1))
        gpool = ctx.enter_context(tc.tile_pool(name="g", bufs=6 if not ant else 2))
        lpool = ctx.enter_context(tc.tile_pool(name="lhs", bufs=3))
        ppool = ctx.enter_context(tc.tile_pool(name="ps", bufs=2, space="PSUM"))
        rpool = ctx.enter_context(tc.tile_pool(name="root", bufs=3))
        epool = ctx.enter_context(tc.tile_pool(name="epi", bufs=2))

        consts_sb = cpool.tile([P, W], F32)
        nc.sync.dma_start(consts_sb[:], consts[:, :])
        idx_area = consts_sb[:, 0:IDXW]
        srcs_sb = None if ant else idx_area.bitcast(I32)
        typesf_sb = consts_sb[:, IDXW : IDXW + NB]
        dstf_sb = consts_sb[:, IDXW + NB : IDXW + 2 * NB]
        cnt_sb = consts_sb[:, IDXW + 2 * NB : IDXW + 2 * NB + T]
        iota_t = consts_sb[:, IDXW + 2 * NB + T : IDXW + 2 * NB + T + IOTA_W]
        if cfg.tdt == "bf16":
            iota_t = iota_t.bitcast(BF16)
        biast_sb = consts_sb[
            :, IDXW + 2 * NB + T + IOTA_W : IDXW + 2 * NB + T + IOTA_W + OUT]
        rcnt_sb = cpool.tile([P, T], F32)
        nc.vector.tensor_scalar_max(rcnt_sb[:], cnt_sb, 1.0)
        nc.vector.reciprocal(rcnt_sb[:], rcnt_sb[:])

        # Per-edge basis coefficients computed once on-device from edge types:
        # c_all[e, b] = comp[type_e, b] = sum_r (type_e == r) * comp[r, b];
        # comp values are baked in as immediates (known at trace time).
        c_all = cpool.tile([P, R, NB], F32)
        ctmp = cpool.tile([P, NB], F32)
        for b in range(R):
            for r in range(R):
                dst_ap = c_all[:, b, :] if r == 0 else ctmp[:]
                nc.vector.tensor_scalar(
                    out=dst_ap, in0=typesf_sb, scalar1=float(r),
                    scalar2=float(comp[r, b]), op0=eq, op1=mul,
                )
                if r > 0:
                    nc.vector.tensor_tensor(
                        out=c_all[:, b, :], in0=c_all[:, b, :], in1=ctmp[:],
                        op=add,
                    )

        # group cells by tile, preserving stream order
        by_tile = [[] for _ in range(T)]
        jstart = 0
        for (t, h, Bc) in cells:
            by_tile[t].append((h, Bc, jstart))
            jstart += Bc

        for t in range(T):
            tile_cells = by_tile[t]
            n_blocks = sum(Bc for (_, Bc, _) in tile_cells)
            psum_t = ppool.tile([P, OUT], F32)
            root_t = rpool.tile([P, OUT], F32)
            nc.sync.dma_start(root_t[:], rootp[t * P : (t + 1) * P, :])
            done = 0
            for (h, Bc, jbase) in tile_cells:
                if ant:
                    gt = gpool.tile([P, Bc, ROW], TDT, tag="g")
                    hi = min((h + 1) * HALF, N)
                    nc.gpsimd.dma_gather(
                        out_ap=gt[:, :, :],
                        in_ap=basisR[h * HALF : hi, :],
                        idxs_ap=idx_area[:, jbase * 4 : (jbase + Bc) * 4].bitcast(I16),
                        num_idxs=Bc * P,
                        num_idxs_reg=Bc * P,
                        elem_size=ROW,
                    )
                for j in range(Bc):
                    col = jbase + j
                    if not ant:
                        gtb = gpool.tile([P, ROW], TDT, tag="g")
                        nc.gpsimd.indirect_dma_start(
                            out=gtb[:, :], out_offset=None,
                            in_=basisR[:, :],
                            in_offset=bass.IndirectOffsetOnAxis(
                                ap=srcs_sb[:, col : col + 1], axis=0),
                        )
                    lt = lpool.tile([P, R, P], TDT, tag="l")
                    for b in range(R):
                        # lhsT_b[e, nd] = (iota[nd] == dst_e) * comp[type_e, b]
                        nc.vector.tensor_scalar(
                            out=lt[:, b, :], in0=iota_t[:],
                            scalar1=dstf_sb[:, col : col + 1],
                            scalar2=c_all[:, b, col : col + 1],
                            op0=eq, op1=mul,
                        )
                        rhs = (gt[:, j, b * OUT : (b + 1) * OUT] if ant
                               else gtb[:, b * OUT : (b + 1) * OUT])
                        nc.tensor.matmul(
                            psum_t[:],
                            lhsT=lt[:, b, :],
                            rhs=rhs,
                            start=(done == 0),
                            stop=(done == n_blocks * R - 1),
                        )
                        done += 1

            # epilogue: mean, +root, +bias, relu, log_softmax, store
            h = epool.tile([P, OUT], F32, tag="h")
            nc.vector.tensor_scalar(
                out=h[:], in0=psum_t[:], scalar1=rcnt_sb[:, t : t + 1],
                scalar2=None, op0=mul,
            )
            nc.vector.tensor_tensor(out=h[:], in0=h[:], in1=root_t[:], op=add)
            nc.vector.tensor_tensor(out=h[:], in0=h[:], in1=biast_sb[:], op=add)
            nc.vector.tensor_scalar_max(h[:], h[:], 0.0)
            mx = epool.tile([P, 2], F32, tag="mx")
            nc.vector.tensor_reduce(
                out=mx[:, 0:1], in_=h[:], axis=mybir.AxisListType.X,
                op=mybir.AluOpType.max, negate=True,
            )
            ex = epool.tile([P, OUT], F32, tag="ex")
            nc.scalar.activation(
                out=ex[:], in_=h[:], func=mybir.ActivationFunctionType.Exp,
                bias=mx[:, 0:1], scale=1.0, accum_out=mx[:, 1:2],
            )
            ln = epool.tile([P, 2], F32, tag="ln")
            nc.scalar.activation(
                out=ln[:, 0:1], in_=mx[:, 1:2], func=mybir.ActivationFunctionType.Ln,
            )
            tot = epool.tile([P, 2], F32, tag="tot")
            nc.vector.tensor_tensor(
                out=tot[:, 0:1], in0=mx[:, 0:1], in1=ln[:, 0:1],
                op=mybir.AluOpType.subtract,
            )
            o = epool.tile([P, OUT], F32, tag="o")
            nc.vector.tensor_scalar(
                out=o[:], in0=h[:], scalar1=tot[:, 0:1], scalar2=None,
                op0=add,
            )
            nc.sync.dma_start(out[t * P : (t + 1) * P, :], o[:])
    nc.compile()
    return nc


# ----------------------------------------------------------------------------
# Entry point
# ----------------------------------------------------------------------------

def _run(cfg: Cfg, inputs: dict, trace: bool = False):
    B_t, in_maps = host_prep(
        cfg,
        inputs["edge_index"], inputs["edge_type"], inputs["basis"],
        inputs["root"], inputs["bias"], inputs["comp"],
    )
    nc = build_program(cfg, B_t, inputs["comp"])
    res = run_bass_kernel_spmd(
        nc, in_maps, core_ids=list(range(cfg.n_cores)), trace=trace,
    )
    parts = [res.results[c]["out"] for c in range(cfg.n_cores)]
    full = np.concatenate(parts, axis=0)[: cfg.n_nodes]
    return np.ascontiguousarray(full.astype(np.float32)), res


def kernel(**inputs) -> np.ndarray:
    cfg = Cfg()
    out, _ = _run(cfg, inputs)
    return out
